# revision 1
# baseline (speedup 1.0000x reference)
"""BlockSparseLocallyConnected forward on 8 Trainium2 NeuronCores.

Data-parallel over batch: 8 images per core, weights replicated.

out[b, nr, nc] = sum_{dr,dc} xpad[b, 16*nr+dr, 16*nc+dc] * w[(nr,nc), dr*32+dc] + bias

Decomposition: dr = 16*h + u, dc = 16*i + v (h,i in {0,1}; u,v in [0,16)),
nr = 8*g + j (g in [0,4), j in [0,8)).  Patch row = 128*g + 16*(j+h) + u.
With two row-shifted copies of the padded image (shift 0 / 16 rows), SBUF
partition p = 16*j + u holds exactly the rows needed, for both h values.
Columns 16*(nc+i)+v are free-dim strides (overlapping AP reads).

Per (b, g): DVE tensor_mul (bf16) -> product [128, (h,nc,i,v)=2048].
PE matmul with 0/1 selector lhsT L_g[16j+u, 8g+j] reduces u over partitions
and accumulates (g, h) into PSUM [128, (nc,i,v)], 4 batches per PSUM tile
(col-tile offsets 0/32/64/96).  DVE tensor_reduce(axis=XY) folds (i,v),
then bias add.  All layout shuffles/casts are host-side numpy so every DMA
is a contiguous 1:1 copy.
"""

import os
import sys

sys.path.insert(0, "/opt/trn_rl_repo")

import numpy as np
import ml_dtypes

# ---- problem constants (hardcoded; kernel.py must be self-contained) ----
B = 64            # batch
H = W = 512
PH = PW = 8
FULL = 528        # padded H/W
NKH = NKW = 32    # window grid
NCORES = 8
BL = B // NCORES  # batches per core = 8
G = 4             # window-row groups of 8 (nr = 8g + j)
WAVES = BL // 4   # psum waves per core = 2

BF16 = ml_dtypes.bfloat16

_CACHE = {}

TRACE = False          # test.py sets True to get exec_time_ns
LAST_RESULTS = None    # BassKernelResults of last run (for test.py)


def _build_program():
    import concourse.bass as bass
    import concourse.bacc as bacc
    import concourse.tile as tile
    from concourse import mybir

    dt_c = mybir.dt.bfloat16
    f32 = mybir.dt.float32

    # Bacc (not plain Bass): its compile() runs generate_event_semaphores,
    # which splits multi-wait instructions (TRN2 allows 1 wait/instruction).
    nc = bacc.Bacc(
        "TRN2", target_bir_lowering=False, debug=False, num_devices=NCORES
    )
    xs = nc.dram_tensor("xs", [BL, G, 128, 2, FULL], dt_c, kind="ExternalInput")
    wp = nc.dram_tensor("wp", [128, G, 2, 2, 32, 16], dt_c, kind="ExternalInput")
    lm = nc.dram_tensor("lm", [128, G, 32], dt_c, kind="ExternalInput")
    bp = nc.dram_tensor("bp", [128, 32], f32, kind="ExternalInput")
    out_d = nc.dram_tensor("out", [WAVES, 128, 32], f32, kind="ExternalOutput")

    with tile.TileContext(nc) as tc:
        with (
            tc.tile_pool(name="xpool", bufs=BL * G) as xpool,
            tc.tile_pool(name="cst", bufs=1) as cst,
            tc.tile_pool(name="ppool", bufs=6) as ppool,
            tc.tile_pool(name="psum", bufs=2, space="PSUM") as psum,
            tc.tile_pool(name="opool", bufs=4) as opool,
        ):
            # ONE ring (SP), strict FIFO, interleaved in exact consumption
            # order — a second competing ring starves the small-packet W
            # transfers (per-packet round-robin) and stalls the stream.
            # x is loaded as per-(b,g) tiles so each product's dependency is
            # a single 270KB transfer.
            l_sb = cst.tile([128, G, 32], dt_c)
            nc.sync.dma_start(out=l_sb[:], in_=lm[:])
            w_all = cst.tile([128, G, 2, 2, 32, 16], dt_c)
            b_sb = cst.tile([128, 32], f32)
            x_sb = [[None] * G for _ in range(BL)]
            for b in range(BL):
                for g in range(G):
                    x_sb[b][g] = xpool.tile(
                        [128, 2, FULL], dt_c, tag="xb", name=f"xb_{b}_{g}"
                    )
            # g-major: each 0.5MB W chunk amortizes over all 8 batches, so
            # the product stream needs only 0.33MB/product of DMA (vs 0.44
            # MB/us delivered) and is never delivery-paced.
            for g in range(G):
                nc.sync.dma_start(out=w_all[:, g], in_=wp[:, g])
                for b in range(BL):
                    nc.sync.dma_start(out=x_sb[b][g][:], in_=xs[b, g])
            nc.sync.dma_start(out=b_sb[:], in_=bp[:])

            # PE warmup during the DMA ramp: ~5us of back-to-back matmuls
            # flips HAM to K=8/8 right before the real matmuls arrive
            # (PE would otherwise run its first ~25us at 1.2GHz and
            # backpressure the DVE product stream).
            warm = cst.tile([128, 512], dt_c)
            nc.vector.memset(warm[:], 1.0)
            wpsum = psum.tile([128, 512], f32, tag="warm")
            for _ in range(12):
                nc.tensor.matmul(wpsum[:], warm[:, 0:128], warm[:],
                                 start=True, stop=True)

            ps_tiles = [
                psum.tile([128, 32, 16], f32, tag="acc", name=f"acc{w}")
                for w in range(WAVES)
            ]
            for g in range(G):
                # one product per (b, g): free = (shift, i, nc*16+v); the
                # single L_g load amortizes over its 4 matmuls.
                for b in range(BL):
                    wv, c = divmod(b, 4)
                    psum_t = ps_tiles[wv]
                    base = x_sb[b][g][:]
                    xview = bass.AP(
                        tensor=base.tensor,
                        offset=base.offset,
                        ap=[
                            list(base.ap[0]),   # partition
                            [FULL, 2],          # shift
                            [16, 2],            # i (col offset 16i)
                            [1, 512],           # nc*16+v contiguous
                        ],
                    )
                    wview = bass.AP(
                        tensor=w_all.tensor,
                        offset=w_all.offset + g * 2048,
                        ap=[
                            list(w_all.ap[0]),  # partition
                            [1024, 2],          # h (=shift)
                            [512, 2],           # i
                            [1, 512],           # nc*16+v
                        ],
                    )
                    prod = ppool.tile([128, 2, 2, 512], dt_c, tag="prod")
                    nc.vector.tensor_mul(prod[:], xview, wview)
                    for i in range(2):
                        for s in range(2):
                            nc.tensor.matmul(
                                psum_t[32 * c : 32 * c + 32, :, :],
                                l_sb[:, g, :],
                                prod[:, s, i],
                                start=(g == 0 and i == 0 and s == 0),
                                stop=(g == G - 1 and i == 1 and s == 1),
                                tile_position=(0, 32 * c),
                            )
            for wv in range(WAVES):
                tmp = opool.tile([128, 32], f32, tag="tmp")
                nc.vector.tensor_reduce(
                    tmp[:], ps_tiles[wv][:],
                    axis=mybir.AxisListType.X, op=mybir.AluOpType.add,
                )
                ow = opool.tile([128, 32], f32, tag="ow")
                nc.vector.tensor_add(ow[:], tmp[:], b_sb[:])
                nc.scalar.dma_start(out=out_d[wv], in_=ow[:])
    nc.compile()
    return nc


def _prep_inputs(x, weight, bias):
    """Host-side packing: pad, row-shift duplicate, (j,u)-major weight shuffle,
    bf16 cast.  Returns per-core in_maps."""
    x = np.asarray(x, dtype=np.float32)
    weight = np.asarray(weight, dtype=np.float32)
    bias = np.asarray(bias, dtype=np.float32)

    xp = np.zeros((B, FULL, FULL), dtype=np.float32)
    xp[:, PH : PH + H, PW : PW + W] = x[:, 0]
    a = xp[:, 0:512, :].reshape(B, G, 128, FULL)
    bshift = xp[:, 16:528, :].reshape(B, G, 128, FULL)
    # (B, 2, G, 128, FULL) -> (B, G, 128, 2, FULL): per-(b,g) slice is a
    # fully contiguous [128, 2*FULL] block (one descriptor per partition)
    xs = np.stack([a, bshift], axis=1).transpose(0, 2, 3, 1, 4)
    xs = np.ascontiguousarray(xs).astype(BF16)

    # weight[(8g+j)*32+nc, (16h+u)*32+16i+v] -> wp[16j+u, g, h, i, nc, v]
    wr = weight.reshape(G, 8, 32, 2, 16, 2, 16)          # (g, j, nc, h, u, i, v)
    wp = wr.transpose(1, 4, 0, 3, 5, 2, 6)               # (j, u, g, h, i, nc, v)
    wp = np.ascontiguousarray(wp.reshape(128, G, 2, 2, 32, 16)).astype(BF16)

    # selector matrices: L[16j+u, g, 8g+j] = 1
    lmat = np.zeros((128, G, 32), dtype=np.float32)
    jj = np.arange(8)
    for g in range(G):
        for j in range(8):
            lmat[16 * j : 16 * j + 16, g, 8 * g + j] = 1.0
    lm = lmat.astype(BF16)

    bpk = np.ascontiguousarray(np.tile(bias.reshape(32, 32), (4, 1)))  # [128, 32]

    in_maps = []
    for k in range(NCORES):
        in_maps.append(
            {
                "xs": np.ascontiguousarray(xs[k * BL : (k + 1) * BL]),
                "wp": wp,
                "lm": lm,
                "bp": bpk,
            }
        )
    return in_maps


def kernel(x, weight, bias):
    global LAST_RESULTS
    from concourse.bass_utils import run_bass_kernel_spmd

    if "nc" not in _CACHE:
        _CACHE["nc"] = _build_program()
    nc = _CACHE["nc"]

    in_maps = _prep_inputs(x, weight, bias)
    res = run_bass_kernel_spmd(
        nc, in_maps, core_ids=list(range(NCORES)), trace=TRACE
    )
    LAST_RESULTS = res
    outs = [r["out"].reshape(BL, NKH, NKW) for r in res.results]
    return np.concatenate(outs, axis=0).astype(np.float32)



# revision 2
# speedup vs baseline: 1.4570x; 1.4570x over previous
"""BlockSparseLocallyConnected forward on 8 Trainium2 NeuronCores.

Window-column shard: core k owns output columns nc in {4k..4k+3}, all 64
batches.  The PE does the real MACs (the DVE tensor_tensor path is capped
at 2x = 34us/core; the PE stream floor is 27.3us/core):

  out[b, nr, nc] = sum_{dr,dc} xpad[b, 16nr+dr, 16nc+dc] * w[nr*32+nc, dr*32+dc]

Contraction (dr, dc) is split into 8 chunks q=(qr, hc) of 128 = (dr_local 8,
c16 16); SBUF partition p = 16*dr_local + c16 holds x rows r = dr_local
(mod 8), cols c = c16 (mod 16) -- window columns start at multiples of 16,
so ONE copy of x serves every (nc, hc) with a pure free-dim offset.  Rows
are stored per partition as [b, m', par, idx] with r = 16*idx + 8*par +
dr_local, so the moving AP for window-row nr_x is contiguous (stride 1).

Per (nc_local j, q): lhsT = weights [128, 32 nr_w] (stationary), rhs = x
[128, (b 16, nr_x 32) = 512] (moving), accumulated over the 8 q-chunks into
PSUM[32j:32j+32, 512] via tile_position=(0, 32j).  The matmul computes all
(nr_w, nr_x) cross terms; only the diagonal nr_w == nr_x is the real
output.  ACT evacuates PSUM -> SBUF adding the per-partition bias, the
full [128, 512] tiles DMA out, and the host gathers the diagonal (free).
"""

import sys

sys.path.insert(0, "/opt/trn_rl_repo")

import numpy as np
import ml_dtypes

# ---- problem constants (hardcoded; kernel.py must be self-contained) ----
B = 64            # batch
H = W = 512
PH = PW = 8
FULL = 528        # padded H/W
NKH = NKW = 32    # window grid
NCORES = 8
NCL = 4           # window-columns per core
FQ = 4            # f-dim chunks (16 batches each)
BFQ = B // FQ     # 16
M = 5             # 16-col blocks per core span (80 cols)

BF16 = ml_dtypes.bfloat16

_CACHE = {}

TRACE = False          # test.py sets True to get exec_time_ns
LAST_RESULTS = None    # BassKernelResults of last run (for test.py)


def _build_program():
    import concourse.bass as bass
    import concourse.bacc as bacc
    import concourse.tile as tile
    from concourse import mybir

    dt_c = mybir.dt.bfloat16
    f32 = mybir.dt.float32

    nc = bacc.Bacc(
        "TRN2", target_bir_lowering=False, debug=False, num_devices=NCORES
    )
    # x: [fq, m, p, bi, par, idx] -- each (fq, m) slab is one contiguous DMA
    xs = nc.dram_tensor("xs", [FQ, M, 128, BFQ, 2, 33], dt_c, kind="ExternalInput")
    # weights: [p, j, qr, hc, nr_w]
    wp = nc.dram_tensor("wp", [128, NCL, 4, 2, 32], dt_c, kind="ExternalInput")
    bp = nc.dram_tensor("bp", [128, 1], f32, kind="ExternalInput")
    out_d = nc.dram_tensor("out", [FQ, 128, 512], f32, kind="ExternalOutput")

    with tile.TileContext(nc) as tc:
        with (
            tc.tile_pool(name="xpool", bufs=FQ * M) as xpool,
            tc.tile_pool(name="cst", bufs=1) as cst,
            tc.tile_pool(name="psum", bufs=2, space="PSUM") as psum,
            tc.tile_pool(name="opool", bufs=4) as opool,
        ):
            # One ring, strict FIFO, in exact consumption order.
            w_sb = cst.tile([128, NCL, 4, 2, 32], dt_c)
            nc.sync.dma_start(out=w_sb[:], in_=wp[:])
            b_sb = cst.tile([128, 1], f32)
            nc.sync.dma_start(out=b_sb[:], in_=bp[:])
            x_sb = [[None] * M for _ in range(FQ)]
            for fq in range(FQ):
                for m in range(M):
                    x_sb[fq][m] = xpool.tile(
                        [128, BFQ, 2, 33], dt_c, tag="xb", name=f"xb_{fq}_{m}"
                    )
                    nc.sync.dma_start(out=x_sb[fq][m][:], in_=xs[fq, m])

            # PE warmup during the DMA ramp: back-to-back matmuls push HAM
            # toward full clock before the real stream starts.
            warm = cst.tile([128, 512], dt_c)
            nc.vector.memset(warm[:], 1.0)
            wpsum = psum.tile([128, 512], f32, tag="warm")
            for _ in range(6):
                nc.tensor.matmul(wpsum[:], warm[:, 0:128], warm[:],
                                 start=True, stop=True)

            # Real stream: per fq, 8 q-chunks x 4 j = 32 matmuls into one
            # PSUM bank.  Ordered by s = j + hc so each matmul only needs
            # x slab m' = j + hc (DMA arrives m-ascending).
            order = []  # (s, j, hc)
            for j in range(NCL):
                order.append((j + 0, j, 0))
                order.append((j + 1, j, 1))
            order.sort()
            for fq in range(FQ):
                ps = psum.tile([128, 512], f32, tag="acc", name=f"acc{fq}")
                seen = [0] * NCL
                for s, j, hc in order:
                    for qr in range(4):
                        xt = x_sb[fq][j + hc][:]
                        rhs = bass.AP(
                            tensor=xt.tensor,
                            offset=xt.offset + 33 * (qr & 1) + (qr >> 1),
                            ap=[
                                list(xt.ap[0]),  # partition
                                [66, BFQ],       # b
                                [1, 32],         # nr_x
                            ],
                        )
                        nc.tensor.matmul(
                            ps[32 * j: 32 * j + 32, :],
                            w_sb[:, j, qr, hc, :],
                            rhs,
                            start=(seen[j] == 0),
                            stop=(seen[j] == 7),
                            tile_position=(0, 32 * j),
                        )
                        seen[j] += 1
                ev = opool.tile([128, 512], f32, tag="ev", name=f"ev{fq}")
                nc.scalar.activation(
                    out=ev[:], in_=ps[:],
                    func=mybir.ActivationFunctionType.Identity,
                    bias=b_sb[:], scale=1.0,
                )
                nc.scalar.dma_start(out=out_d[fq], in_=ev[:])
    nc.compile()
    return nc


def _prep_inputs(x, weight, bias):
    """Host-side packing into the transposed (mod-8 row, mod-16 col)
    partition layout; bf16 cast.  Returns per-core in_maps."""
    x = np.asarray(x, dtype=np.float32)
    weight = np.asarray(weight, dtype=np.float32)
    bias = np.asarray(bias, dtype=np.float32)

    xpad = np.zeros((B, FULL, FULL), dtype=np.float32)
    xpad[:, PH:PH + H, PW:PW + W] = x[:, 0]
    xpb = xpad.astype(BF16)

    # r = 16*idx + 8*par + dl
    dl = np.arange(8)[:, None, None]
    par = np.arange(2)[None, :, None]
    idx = np.arange(33)[None, None, :]
    r_map = 16 * idx + 8 * par + dl                      # [8, 2, 33]

    w4 = weight.reshape(32, 32, 32, 32)                  # [nr, nc, dr, dc]
    bv = bias.reshape(32, 32)                            # [nr, nc]

    in_maps = []
    for k in range(NCORES):
        c_map = (16 * (4 * k + np.arange(M))[:, None]
                 + np.arange(16)[None, :])               # [m, c16]
        # gather -> [b, dl, par, idx, m, c16]
        g = xpb[:, r_map.reshape(8, 2, 33, 1, 1),
                c_map.reshape(1, 1, 1, M, 16)]
        # -> [fq, bi, dl, par, idx, m, c16]
        g = g.reshape(FQ, BFQ, 8, 2, 33, M, 16)
        # -> [fq, m, dl, c16, bi, par, idx]
        g = g.transpose(0, 5, 2, 6, 1, 3, 4)
        xs = np.ascontiguousarray(g.reshape(FQ, M, 128, BFQ, 2, 33))

        # weights: [nr, j, qr, dl, hc, c16] -> [dl, c16, j, qr, hc, nr]
        wk = w4[:, 4 * k:4 * k + NCL].reshape(32, NCL, 4, 8, 2, 16)
        wk = wk.transpose(3, 5, 1, 2, 4, 0)
        wpk = np.ascontiguousarray(wk.reshape(128, NCL, 4, 2, 32)).astype(BF16)

        # bias: partition 32j + nr_w -> bias[nr_w, 4k+j]
        bk = np.ascontiguousarray(
            bv[:, 4 * k:4 * k + NCL].T.reshape(128, 1))

        in_maps.append({"xs": xs, "wp": wpk, "bp": bk})
    return in_maps


def kernel(x, weight, bias):
    global LAST_RESULTS
    from concourse.bass_utils import run_bass_kernel_spmd

    if "nc" not in _CACHE:
        _CACHE["nc"] = _build_program()
    nc = _CACHE["nc"]

    in_maps = _prep_inputs(x, weight, bias)
    res = run_bass_kernel_spmd(
        nc, in_maps, core_ids=list(range(NCORES)), trace=TRACE
    )
    LAST_RESULTS = res

    out = np.empty((B, NKH, NKW), dtype=np.float32)
    ar = np.arange(32)
    for k in range(NCORES):
        r5 = res.results[k]["out"].reshape(FQ, NCL, 32, BFQ, 32)
        d = r5[:, :, ar, :, ar]                 # [nr, fq, j, bi]
        d = d.transpose(1, 3, 0, 2)             # [fq, bi, nr, j]
        out[:, :, 4 * k:4 * k + NCL] = d.reshape(B, NKH, NCL)
    return out
